# revision 24
# baseline (speedup 1.0000x reference)
"""Trainium2 Bass kernel for nn_CE_25872882991735.

Reference computation (per full batch X [N=32, C=256, H=64, W=64]):
  AR branch:  x_var[n,c] (unbiased over spatial) -> MLP+LN+sigmoid -> y[n,c]
              scale = sqrt(mean(x_var));  xin = (y/scale) * X
  Whitening:  Sigma[g] = I/m + EPS * xc@xc^T  (G=4 groups of d=64 channels,
              m = N*H*W), Newton-Schulz T=3 -> P[g];  Xn = P @ x (uncentered)
  out = w*Xn + (1-w)*xin,  w = sigmoid(x_weight)

Distribution: data-parallel over batch N across 8 cores (4 images each).
Per-core pipeline:
  Phase 1: stream X in (f32), ACT-cast to a resident bf16 copy (row sums
           accumulated in the same pass), per-(n,half) Gram matrices via
           regular matmuls against identity (counts as PE activity for the
           HAM clock gate, unlike transpose-mode) + bf16 matmul accumulation.
  TWO AllReduces: AR0 ships the h=0 Sigma partials as soon as the first four
  tiles finish, so its transport AND the h=0 Sigma/Newton chain hide under
  the h=1 half of phase 1.  AR1 ships h=1 partials + x_var stats at the end.
  Local MLP for y overlaps AR1; Newton iterations run in bf16.
  Phase 2: per tile a single fused bf16 matmul
           out[n, half] = (w*P_half + diag((1-w)*y[n]/scale)) @ X_bf16[n, half]
           stored as bf16 (upcast to f32 on host).
"""
import sys

try:
    import concourse.bass as bass  # noqa: F401
except ImportError:  # pragma: no cover
    sys.path.insert(0, "/opt/trn_rl_repo")

import numpy as np

import concourse.bacc as bacc
import concourse.tile as tile
from concourse import mybir
from concourse import bass_utils

F32 = mybir.dt.float32
BF16 = mybir.dt.bfloat16
AX = mybir.AxisListType
ALU = mybir.AluOpType
ACTF = mybir.ActivationFunctionType

N_CORES = 8
EPS = 1e-5
LN_EPS = 1e-5
T_NEWTON = 3

# ---- packed constants column layouts ----
# "crit" is tiny (consumers get hoisted to engine-queue heads by the
# scheduler, so it must land within a few us of kernel start); "rest" is
# only consumed mid/late-kernel and is deferred behind tile-1's loads.
_CRIT_COLS = {}
_REST_COLS = {}


def _build_cols():
    c = 0
    for name, w in [("ident", 128), ("onesrow", 128), ("xw", 1)]:
        _CRIT_COLS[name] = (c, c + w)
        c += w
    cw_crit = c
    c = 0
    for name, w in [("fc1t", 128), ("fc2t", 256), ("gmask", 2),
                    ("gmaskT15", 128), ("ones", 1), ("lng", 64),
                    ("lnb", 64)]:
        _REST_COLS[name] = (c, c + w)
        c += w
    return cw_crit, c


CW_CRIT, CW_REST = _build_cols()


def _consts_pack(fc1_w, fc2_w, ln_g, ln_b, x_weight, m_total):
    """Host-side: pack constants + small weights into two [128, W] f32."""
    cpc = np.zeros((128, CW_CRIT), np.float32)
    cpr = np.zeros((128, CW_REST), np.float32)

    def putc(name, arr):
        c0, c1 = _CRIT_COLS[name]
        cpc[:arr.shape[0], c0:c1] = arr

    def putr(name, arr):
        c0, c1 = _REST_COLS[name]
        cpr[:arr.shape[0], c0:c1] = arr

    ident = np.eye(128, dtype=np.float32)
    putc("ident", ident)
    putc("onesrow", np.ones((1, 128), np.float32))
    putc("xw", np.asarray(x_weight, np.float32).reshape(1, 1))

    f1 = np.ascontiguousarray(fc1_w.T).reshape(2, 128, 64)
    f1p = np.zeros((128, 128), np.float32)
    f1p[:, 0:64] = f1[0]
    f1p[:, 64:128] = f1[1]
    putr("fc1t", f1p)
    f2 = np.zeros((64, 256), np.float32)
    f2[:, :] = fc2_w.T
    putr("fc2t", f2)
    gmask = np.zeros((128, 2), np.float32)
    gmask[:64, 0] = 1.0
    gmask[64:, 1] = 1.0
    putr("gmask", gmask)
    putr("gmaskT15", (1.5 * gmask.T).astype(np.float32))
    putr("ones", np.ones((128, 1), np.float32))
    # ln_g / ln_b replicated on partitions 0..3 (per-image LN operands)
    putr("lng", np.tile(np.asarray(ln_g, np.float32).reshape(1, 64), (4, 1)))
    putr("lnb", np.tile(np.asarray(ln_b, np.float32).reshape(1, 64), (4, 1)))
    return cpc, cpr


def build_kernel(n_local=4, S=4096, n_cores=N_CORES):
    """Build the per-core SPMD kernel. S = H*W spatial size per image."""
    C = 256
    NK = n_local * 2          # number of [128, S] tiles (h x n), h-major
    SC = S // 512             # 512-col chunks per tile
    m_total = n_cores * n_local * S
    n_total_imgs = n_cores * n_local

    nc = bacc.Bacc("TRN2", target_bir_lowering=False, num_devices=n_cores)

    Xd = nc.declare_dram_parameter("X", [n_local, 2, 128, S], F32, isOutput=False)
    outd = nc.declare_dram_parameter("out", [n_local, 2, 128, S], BF16, isOutput=True)
    cpcd = nc.declare_dram_parameter("cpack_crit", [128, CW_CRIT], F32,
                                     isOutput=False)
    cprd = nc.declare_dram_parameter("cpack_rest", [128, CW_REST], F32,
                                     isOutput=False)

    with tile.TileContext(nc) as tc:
        _build_tile(tc, Xd, outd, cpcd, cprd, n_local=n_local, S=S,
                    n_cores=n_cores, C=C, NK=NK, SC=SC, m_total=m_total,
                    n_total_imgs=n_total_imgs)
    nc.finalize()
    return nc


def _build_tile(tc, Xd, outd, cpcd, cprd, *, n_local, S, n_cores, C, NK, SC,
                m_total, n_total_imgs):
    nc = tc.nc
    from contextlib import ExitStack
    ctx = ExitStack()
    with ctx:
        consts = ctx.enter_context(tc.tile_pool(name="consts", bufs=1))
        xb_pool = ctx.enter_context(tc.tile_pool(name="xb", bufs=1))
        stats = ctx.enter_context(tc.tile_pool(name="stats", bufs=1))
        stage_pool = ctx.enter_context(tc.tile_pool(name="stage", bufs=3))
        scr_pool = ctx.enter_context(tc.tile_pool(name="scr", bufs=2))
        small = ctx.enter_context(tc.tile_pool(name="small", bufs=1))
        dram = ctx.enter_context(tc.tile_pool(name="dram", bufs=1, space="DRAM"))

        # ---- constants ----
        cpc = consts.tile([128, CW_CRIT], F32)
        nc.sync.dma_start(out=cpc[:], in_=cpcd[:, :])
        cpr = consts.tile([128, CW_REST], F32)

        def csc(name, rows=128):
            c0, c1 = _CRIT_COLS[name]
            return cpc[0:rows, c0:c1]

        def cs(name, rows=128):
            c0, c1 = _REST_COLS[name]
            return cpr[0:rows, c0:c1]

        ident = csc("ident")
        onesrow = csc("onesrow", rows=1)
        xw = csc("xw", rows=1)
        fc1t = cs("fc1t")
        fc2t = cs("fc2t", rows=64)
        gmask = cs("gmask")
        gmaskT15 = cs("gmaskT15", rows=2)
        ones = cs("ones")
        lng4 = cs("lng", rows=n_local)
        lnb4 = cs("lnb", rows=n_local)

        ident_bf = consts.tile([128, 128], BF16)
        nc.vector.tensor_copy(ident_bf[:], ident)
        # derived mask constants (cheap on-chip; keeps the DMA packs small)
        maskeps = consts.tile([128, 128], F32)
        nc.vector.memset(maskeps[:], 0.0)
        nc.vector.memset(maskeps[0:64, 0:64], EPS)
        nc.vector.memset(maskeps[64:128, 64:128], EPS)
        iov = consts.tile([128, 128], F32)
        nc.vector.tensor_scalar(out=iov[:], in0=ident, scalar1=1.0 / m_total,
                                scalar2=None, op0=ALU.mult)
        neghalf_bf = consts.tile([128, 128], BF16)
        nc.vector.tensor_scalar(out=neghalf_bf[:], in0=ident, scalar1=-0.5,
                                scalar2=None, op0=ALU.mult)
        # w = sigmoid(x_weight); onemw = 1 - w ; wcol broadcast (all early)
        w_sb = small.tile([1, 1], F32)
        nc.scalar.activation(out=w_sb[:], in_=xw, func=ACTF.Sigmoid)
        onemw = small.tile([1, 1], F32)
        nc.vector.tensor_scalar(out=onemw[:], in0=w_sb[:], scalar1=-1.0,
                                scalar2=1.0, op0=ALU.mult, op1=ALU.add)

        # ---- stats tiles ----
        rs = stats.tile([128, NK], F32)    # rowsums per (h,n)
        rsa = stats.tile([128, NK], F32)
        rsb = stats.tile([128, NK], F32)
        ss = stats.tile([128, NK], F32)    # sum of squares per (h,n)
        xv = stats.tile([128, NK], F32)    # x_var per (h,n)

        PAY0W = 66   # [h0 blocks | chs0 | dsum0]
        PAY1W = 67   # [h1 blocks | chs1 | dsum1 | xvsum]
        ccin = [dram.tile([128, PAY0W], F32, tag="cc0", name="ccin0"),
                dram.tile([128, PAY1W], F32, tag="cc1", name="ccin1")]
        ccout = [dram.tile([128, PAY0W], F32, addr_space="Shared", tag="cco0",
                           name="ccout0"),
                 dram.tile([128, PAY1W], F32, addr_space="Shared", tag="cco1",
                           name="ccout1")]
        gpay = [small.tile([128, PAY0W], F32, tag="gpay0", name="gpay0"),
                small.tile([128, PAY1W], F32, tag="gpay1", name="gpay1")]

        mw = [small.tile([128, 128], F32, tag=f"mw{h}", name=f"mw{h}")
              for h in range(2)]
        wcol = small.tile([128, 1], F32)

        def pack_and_allreduce(h, payw):
            """Local reductions for half h + payload + AllReduce issue."""
            chs_h = small.tile([128, 1], F32, tag=f"chs{h}", name=f"chs{h}")
            nc.vector.tensor_reduce(chs_h[:], rs[:, n_local * h:n_local * (h + 1)],
                                    axis=AX.X, op=ALU.add)
            sloc = small.tile([128, 128], F32, tag=f"sloc{h}", name=f"sloc{h}")
            nc.vector.tensor_copy(sloc[:], pg[h][:, 0:128])
            for nn_ in range(1, n_local):
                nc.vector.tensor_add(sloc[:], sloc[:],
                                     pg[h][:, 128 * nn_:128 * (nn_ + 1)])
            pay = small.tile([128, payw], F32, tag=f"pay{h}", name=f"pay{h}")
            if payw > 66:
                nc.vector.memset(pay[:, 66:payw], 0.0)
            nc.vector.tensor_copy(pay[0:64, 0:64], sloc[0:64, 0:64])
            nc.vector.tensor_copy(pay[64:128, 0:64], sloc[64:128, 64:128])
            nc.vector.tensor_copy(pay[:, 64:65], chs_h[:])
            nc.vector.tensor_reduce(pay[:, 65:66],
                                    ss[:, n_local * h:n_local * (h + 1)],
                                    axis=AX.X, op=ALU.add)
            return pay

        def sigma_newton(h, sgp):
            """Per-half Sigma assembly + bf16 Newton-Schulz -> mw[h]."""
            gp = gpay[h]
            sglob = small.tile([128, 128], F32, tag=f"sglob{h}",
                               name=f"sglob{h}")
            nc.vector.memset(sglob[:], 0.0)
            nc.vector.tensor_copy(sglob[0:64, 0:64], gp[0:64, 0:64])
            nc.vector.tensor_copy(sglob[64:128, 64:128], gp[64:128, 0:64])
            # traces of the two groups in this half
            rhs2 = small.tile([128, 2], F32, tag=f"rhs2{h}", name=f"rhs2{h}")
            nc.vector.tensor_copy(rhs2[:, 0:1], gp[:, 65:66])
            nc.vector.tensor_mul(rhs2[:, 1:2], gp[:, 64:65], gp[:, 64:65])
            # all small PSUM lives as slices of one rotating 1-bank tile
            Tm = sgp.tile([128, 512], F32, tag="sg", bufs=2, name=f"sgm{h}")
            tr_ps = Tm[0:2, 0:2]
            rtr_ps = Tm[0:128, 8:9]
            chr_ps = Tm[0:1, 16:144]
            u_ps = Tm[0:128, 144:272]
            nc.tensor.matmul(tr_ps, lhsT=gmask, rhs=rhs2[:], start=True,
                             stop=True)
            trg = small.tile([2, 1], F32, tag=f"trg{h}", name=f"trg{h}")
            nc.vector.tensor_scalar(out=trg[:], in0=tr_ps[0:2, 1:2],
                                    scalar1=-1.0 / m_total, scalar2=None,
                                    op0=ALU.mult)
            nc.vector.tensor_add(trg[:], trg[:], tr_ps[0:2, 0:1])
            nc.vector.tensor_scalar(out=trg[:], in0=trg[:], scalar1=EPS,
                                    scalar2=64.0 / m_total, op0=ALU.mult,
                                    op1=ALU.add)
            nc.vector.reciprocal(trg[:], trg[:])
            nc.tensor.matmul(rtr_ps, lhsT=gmaskT15, rhs=trg[:],
                             start=True, stop=True)
            rtrcol = small.tile([128, 1], F32, tag=f"rtc{h}", name=f"rtc{h}")
            nc.vector.tensor_copy(rtrcol[:], rtr_ps)
            # U = chs (x) chs / m
            nc.tensor.transpose(chr_ps, gp[:, 64:65], ident)
            chrow = small.tile([1, 128], F32, tag=f"chr{h}", name=f"chr{h}")
            chrow_m = small.tile([1, 128], F32, tag=f"chm{h}", name=f"chm{h}")
            nc.vector.tensor_copy(chrow[:], chr_ps)
            nc.vector.tensor_scalar(out=chrow_m[:], in0=chr_ps,
                                    scalar1=1.0 / m_total, scalar2=None,
                                    op0=ALU.mult)
            nc.tensor.matmul(u_ps, lhsT=chrow_m[:], rhs=chrow[:],
                             start=True, stop=True)
            sig = small.tile([128, 128], F32, tag=f"sig{h}", name=f"sig{h}")
            nc.vector.tensor_sub(sig[:], sglob[:], u_ps)
            nc.vector.tensor_mul(sig[:], sig[:], maskeps[:])
            nc.vector.tensor_add(sig[:], sig[:], iov[:])
            sig15 = small.tile([128, 128], BF16, tag=f"s15{h}", name=f"s15{h}")
            nc.vector.tensor_scalar(out=sig15[:], in0=sig[:],
                                    scalar1=rtrcol[:], scalar2=None,
                                    op0=ALU.mult)
            # Newton-Schulz in bf16 (P error ~3e-4, way under tolerance)
            P = small.tile([128, 128], BF16, tag=f"P{h}", name=f"P{h}")
            nc.vector.tensor_add(P[:], sig15[:], neghalf_bf[:])
            p2 = small.tile([128, 128], BF16, tag=f"p2{h}", name=f"p2{h}")
            px = small.tile([128, 128], BF16, tag=f"px{h}", name=f"px{h}")
            for it in range(1, T_NEWTON):
                Tn = sgp.tile([128, 512], F32, tag="sg", bufs=2,
                              name=f"sgn{h}{it}")
                ps_a = Tn[0:128, 0:128]
                ps_b = Tn[0:128, 128:256]
                ps_c = Tn[0:128, 256:384]
                nc.tensor.matmul(ps_a, lhsT=P[:], rhs=P[:], start=True,
                                 stop=True)
                nc.tensor.matmul(ps_b, lhsT=P[:], rhs=sig15[:], start=True,
                                 stop=True)
                nc.vector.tensor_copy(p2[:], ps_a)
                nc.vector.tensor_copy(px[:], ps_b)
                nc.tensor.matmul(ps_c, lhsT=p2[:], rhs=px[:],
                                 start=True, stop=False)
                nc.tensor.matmul(ps_c, lhsT=P[:], rhs=neghalf_bf[:],
                                 start=False, stop=True)
                nc.vector.tensor_copy(P[:], ps_c)
            nc.vector.tensor_scalar(out=mw[h][:], in0=P[:], scalar1=wcol[:],
                                    scalar2=None, op0=ALU.mult)

        # ================= PHASE 1 + AR0 + hidden h0 chain =================
        xb_tiles = []
        SH = S // 2
        sg_pool = tc.tile_pool(name="sg_ps", bufs=1, space="PSUM")
        with sg_pool as sgp:
            pg_pool = tc.tile_pool(name="gram", bufs=1, space="PSUM")
            tp_pool = tc.tile_pool(name="tp", bufs=2, space="PSUM")
            chunk_pool = tc.tile_pool(name="chunk", bufs=4)
            with pg_pool as pgp, tp_pool as tpp, chunk_pool as chp:
                pg = [pgp.tile([128, 128 * n_local], F32, tag=f"pg{h}",
                               name=f"pg{h}") for h in range(2)]
                for k in range(NK):
                    h, n = divmod(k, n_local)
                    if k == 2:
                        # bulky consts land by mid-phase-1, behind tile-1
                        # loads so they don't race tile 0
                        nc.sync.dma_start(out=cpr[:], in_=cprd[:, :])
                    xr = xb_pool.tile([128, S], BF16, tag=f"xb{k}")
                    xb_tiles.append(xr)
                    # both halves of a tile ride ONE queue so the transfer
                    # the pipeline needs first completes first
                    ldeng = nc.gpsimd if k % 2 == 0 else nc.sync
                    for half_i, acc in ((0, rsa), (1, rsb)):
                        xin = stage_pool.tile([128, SH], F32, tag="stage",
                                              name=f"xin{k}_{half_i}")
                        ldeng.dma_start(
                            out=xin[:],
                            in_=Xd[n, h][:, SH * half_i:SH * (half_i + 1)])
                        # bf16 cast on ACT, accumulating row sums in one pass
                        nc.scalar.activation(
                            out=xr[:, SH * half_i:SH * (half_i + 1)],
                            in_=xin[:], func=ACTF.Copy,
                            accum_out=acc[:, k:k + 1])
                    for c2 in range(SC // 2):
                        # transpose via REGULAR matmul (lhsT=chunk, rhs=I):
                        # out = chunk^T. Counts as PE activity for the HAM
                        # clock gate (transpose-mode does not), keeping the
                        # array at 2.4 GHz.
                        tp = tpp.tile([128, 1024], F32)
                        for q in range(8):
                            col0 = 1024 * c2 + 128 * q
                            nc.tensor.matmul(
                                tp[:, 128 * q:128 * q + 128],
                                lhsT=xr[:, col0:col0 + 128],
                                rhs=ident_bf[:], start=True, stop=True)
                        chbf = chp.tile([128, 1024], BF16)
                        if c2 == 2:
                            nc.scalar.copy(chbf[:], tp[:])
                        else:
                            nc.vector.tensor_copy(chbf[:], tp[:])
                        for q in range(8):
                            nc.tensor.matmul(
                                pg[h][:, 128 * n:128 * n + 128],
                                lhsT=chbf[:, 128 * q:128 * q + 128],
                                rhs=chbf[:, 128 * q:128 * q + 128],
                                start=(c2 == 0 and q == 0),
                                stop=(c2 == SC // 2 - 1 and q == 7))
                    nc.vector.tensor_add(rs[:, k:k + 1], rsa[:, k:k + 1],
                                         rsb[:, k:k + 1])
                    # diag of Gram -> sum of squares -> x_var for this tile
                    scr = scr_pool.tile([128, 128], F32)
                    nc.vector.tensor_mul(scr[:],
                                         pg[h][:, 128 * n:128 * n + 128],
                                         ident)
                    nc.vector.tensor_reduce(ss[:, k:k + 1], scr[:], axis=AX.X,
                                            op=ALU.add)
                    t1 = scr_pool.tile([128, 1], F32, tag="t1", name=f"xvt{k}")
                    nc.vector.tensor_mul(t1[:], rs[:, k:k + 1], rs[:, k:k + 1])
                    nc.vector.tensor_scalar(
                        out=t1[:], in0=t1[:], scalar1=1.0 / (S * (S - 1.0)),
                        scalar2=None, op0=ALU.mult)
                    nc.vector.tensor_scalar(
                        out=xv[:, k:k + 1], in0=ss[:, k:k + 1],
                        scalar1=1.0 / (S - 1.0), scalar2=None, op0=ALU.mult)
                    nc.vector.tensor_sub(xv[:, k:k + 1], xv[:, k:k + 1], t1[:])

                    if k == n_local - 1:
                        # ---- AR0: h0 Sigma partials, hidden under the h=1
                        # half of phase 1 ----
                        pay0 = pack_and_allreduce(0, PAY0W)
                        nc.sync.dma_start(out=ccin[0][:], in_=pay0[:])
                        nc.gpsimd.collective_compute(
                            "AllReduce", ALU.add,
                            replica_groups=[list(range(n_cores))],
                            ins=[ccin[0][:].opt()], outs=[ccout[0][:].opt()])
                        # wcol broadcast (PE, cheap, deps ready)
                        Tw = sgp.tile([128, 512], F32, tag="sg", bufs=2,
                                      name="wcps")
                        nc.tensor.matmul(Tw[0:128, 0:1], lhsT=onesrow,
                                         rhs=w_sb[:], start=True, stop=True)
                        nc.vector.tensor_copy(wcol[:], Tw[0:128, 0:1])

                # gpay0 readback AFTER all load triggers: the readback waits
                # on AR0 completion, and anything behind it in its queue
                # would stall. The h0 Sigma/Newton chain then hides under
                # AR1's transport.
                nc.gpsimd.dma_start(out=gpay[0][:], in_=ccout[0][:])
                sigma_newton(0, sgp)

                # ---- h1 local reductions + x_var global partial + AR1 ----
                pay1 = pack_and_allreduce(1, PAY1W)
            ssum = small.tile([128, 1], F32)
            nc.vector.tensor_reduce(ssum[:], ss[:], axis=AX.X, op=ALU.add)
            rs2 = small.tile([128, NK], F32)
            nc.vector.tensor_mul(rs2[:], rs[:], rs[:])
            rssum = small.tile([128, 1], F32)
            nc.vector.tensor_reduce(rssum[:], rs2[:], axis=AX.X, op=ALU.add)
            xvr = small.tile([128, 1], F32)
            nc.vector.tensor_scalar(out=xvr[:], in0=rssum[:],
                                    scalar1=-1.0 / (S * (S - 1.0)),
                                    scalar2=None, op0=ALU.mult)
            nc.vector.tensor_scalar(out=rssum[:], in0=ssum[:],
                                    scalar1=1.0 / (S - 1.0), scalar2=None,
                                    op0=ALU.mult)
            nc.vector.tensor_add(xvr[:], xvr[:], rssum[:])
            Txv = sgp.tile([128, 512], F32, tag="sg", bufs=2, name="psxv")
            nc.tensor.matmul(Txv[0:1, 0:1], lhsT=xvr[:], rhs=ones, start=True,
                             stop=True)
            nc.vector.tensor_copy(pay1[0:1, 66:67], Txv[0:1, 0:1])
            nc.sync.dma_start(out=ccin[1][:], in_=pay1[:])
            nc.gpsimd.collective_compute(
                "AllReduce", ALU.add,
                replica_groups=[list(range(n_cores))],
                ins=[ccin[1][:].opt()], outs=[ccout[1][:].opt()])
            nc.gpsimd.dma_start(out=gpay[1][:], in_=ccout[1][:])

            # ======== AR BRANCH MLP (overlaps AR1) ========
            with tc.tile_pool(name="spsum", bufs=2, space="PSUM") as spsum:
                h_ps = spsum.tile([n_local, 64], F32, tag="sp")
                for h in range(2):
                    nc.tensor.matmul(
                        h_ps[:], lhsT=xv[:, n_local * h:n_local * (h + 1)],
                        rhs=fc1t[:, 64 * h:64 * h + 64], start=(h == 0),
                        stop=(h == 1))
                h_sb = small.tile([n_local, 64], F32)
                nc.vector.tensor_copy(h_sb[:], h_ps[:])
                # LayerNorm over the 64 features (ACT sqrt is ~2ULP: no
                # refinement steps needed at 2e-2 tolerance)
                bst = small.tile([n_local, 6], F32)
                nc.vector.bn_stats(out=bst[:], in_=h_sb[:])
                mv = small.tile([n_local, 2], F32)
                nc.vector.bn_aggr(out=mv[:], in_=bst[:])
                ve = small.tile([n_local, 1], F32)
                nc.vector.tensor_scalar(out=ve[:], in0=mv[:, 1:2],
                                        scalar1=LN_EPS, scalar2=None,
                                        op0=ALU.add)
                s0 = small.tile([n_local, 1], F32)
                nc.scalar.activation(out=s0[:], in_=ve[:], func=ACTF.Sqrt)
                rstd = small.tile([n_local, 1], F32)
                nc.vector.reciprocal(rstd[:], s0[:])
                hln = small.tile([n_local, 64], F32)
                nc.vector.tensor_scalar(out=hln[:], in0=h_sb[:],
                                        scalar1=mv[:, 0:1], scalar2=rstd[:],
                                        op0=ALU.subtract, op1=ALU.mult)
                nc.vector.tensor_mul(hln[:], hln[:], lng4)
                nc.vector.tensor_add(hln[:], hln[:], lnb4)
                nc.vector.tensor_scalar_max(hln[:], hln[:], 0.0)
                hT_ps = spsum.tile([64, n_local], F32, tag="sp")
                nc.tensor.transpose(hT_ps[:], hln[:],
                                    ident[0:n_local, 0:n_local])
                hT = small.tile([64, n_local], F32)
                nc.vector.tensor_copy(hT[:], hT_ps[:])
                y_ps = spsum.tile([n_local, 256], F32, tag="sp")
                nc.tensor.matmul(y_ps[:], lhsT=hT[:], rhs=fc2t, start=True,
                                 stop=True)
                y_sb = small.tile([n_local, 256], F32)
                nc.scalar.activation(out=y_sb[:], in_=y_ps[:],
                                     func=ACTF.Sigmoid)
                # transpose y halves -> yT [128, NK] (col k = h*n_local+n)
                yT = small.tile([128, NK], F32)
                for h in range(2):
                    yT_ps = spsum.tile([128, n_local], F32, tag="sp")
                    nc.tensor.transpose(yT_ps[:],
                                        y_sb[:, 128 * h:128 * h + 128],
                                        ident[0:n_local, 0:n_local])
                    nc.vector.tensor_copy(yT[:, n_local * h:n_local * (h + 1)],
                                          yT_ps[:])

            # ======== POST-AR1: scale + h1 Sigma/Newton ========
            xvm = small.tile([1, 1], F32)
            nc.vector.tensor_scalar(out=xvm[:], in0=gpay[1][0:1, 66:67],
                                    scalar1=1.0 / (n_total_imgs * C),
                                    scalar2=None, op0=ALU.mult)
            sq0 = small.tile([1, 1], F32)
            nc.scalar.activation(out=sq0[:], in_=xvm[:], func=ACTF.Sqrt)
            rscale = small.tile([1, 1], F32)
            nc.vector.reciprocal(rscale[:], sq0[:])
            yscs = small.tile([1, 1], F32)
            nc.vector.tensor_mul(yscs[:], onemw[:], rscale[:])
            Tys = sgp.tile([128, 512], F32, tag="sg", bufs=2, name="ysps")
            nc.tensor.matmul(Tys[0:128, 0:1], lhsT=onesrow, rhs=yscs[:],
                             start=True, stop=True)
            yscol = small.tile([128, 1], F32)
            nc.vector.tensor_copy(yscol[:], Tys[0:128, 0:1])
            yT2 = small.tile([128, NK], F32)
            nc.vector.tensor_scalar(out=yT2[:], in0=yT[:], scalar1=yscol[:],
                                    scalar2=None, op0=ALU.mult)
            sigma_newton(1, sgp)

        # ============ PHASE 2: fused bf16 apply ============
        mpool = ctx.enter_context(tc.tile_pool(name="mts", bufs=3))
        dtile_pool = ctx.enter_context(tc.tile_pool(name="dtile", bufs=2))
        ostage_pool = ctx.enter_context(tc.tile_pool(name="ostage", bufs=2))
        st_engs = [nc.sync, nc.gpsimd, nc.scalar]
        with tc.tile_pool(name="apply_ps", bufs=2, space="PSUM") as app:
            for k in range(NK):
                h, n = divmod(k, n_local)
                # M = w*P_h + diag(yscs*y[n]) in bf16
                dtile = dtile_pool.tile([128, 128], F32)
                nc.scalar.activation(out=dtile[:], in_=ident, func=ACTF.Copy,
                                     scale=yT2[:, k:k + 1])
                m_b = mpool.tile([128, 128], BF16)
                nc.vector.tensor_add(m_b[:], dtile[:], mw[h][:])
                ost = ostage_pool.tile([128, S], BF16, tag="ostage",
                                       name=f"ost{k}")
                for half_i in range(2):
                    ap = app.tile([128, 2048], F32)
                    for jj in range(4):
                        c0 = 512 * jj
                        nc.tensor.matmul(
                            ap[:, c0:c0 + 512], lhsT=m_b[:],
                            rhs=xb_tiles[k][:, SH * half_i + c0:SH * half_i + c0 + 512],
                            start=True, stop=True)
                    ocol = SH * half_i
                    nc.vector.tensor_copy(ost[:, ocol:ocol + 1024],
                                          ap[:, 0:1024])
                    nc.scalar.copy(ost[:, ocol + 1024:ocol + 2048],
                                   ap[:, 1024:2048])
                    steng = st_engs[(2 * k + half_i) % 3]
                    steng.dma_start(
                        out=outd[n, h][:, ocol:ocol + SH],
                        in_=ost[:, ocol:ocol + SH])


_KERNEL_CACHE = {}


def _get_kernel(n_local=4, S=4096):
    key = (n_local, S)
    if key not in _KERNEL_CACHE:
        _KERNEL_CACHE[key] = build_kernel(n_local=n_local, S=S)
    return _KERNEL_CACHE[key]


def _make_in_maps(inputs, n_local=4, S=4096):
    X = np.asarray(inputs["X"], dtype=np.float32)
    m_total = X.shape[0] * S
    cpc, cpr = _consts_pack(np.asarray(inputs["fc1_w"], np.float32),
                            np.asarray(inputs["fc2_w"], np.float32),
                            np.asarray(inputs["ln_g"], np.float32),
                            np.asarray(inputs["ln_b"], np.float32),
                            np.asarray(inputs["x_weight"], np.float32),
                            m_total)
    in_maps = []
    for i in range(N_CORES):
        shard = X[i * n_local:(i + 1) * n_local].reshape(n_local, 2, 128, S)
        in_maps.append({"X": np.ascontiguousarray(shard),
                        "cpack_crit": cpc, "cpack_rest": cpr})
    return in_maps


def kernel(X, fc1_w, ln_g, ln_b, fc2_w, x_weight):
    X = np.asarray(X, dtype=np.float32)
    N, C, H, W = X.shape
    assert (N, C, H, W) == (32, 256, 64, 64)
    S = H * W
    n_local = N // N_CORES

    nc = _get_kernel()
    in_maps = _make_in_maps(
        {"X": X, "fc1_w": fc1_w, "ln_g": ln_g, "ln_b": ln_b,
         "fc2_w": fc2_w, "x_weight": x_weight}, n_local=n_local, S=S)

    res = bass_utils.run_bass_kernel_spmd(nc, in_maps,
                                          core_ids=list(range(N_CORES)))
    out = np.empty((N, C, H, W), dtype=np.float32)
    for i in range(N_CORES):
        out[i * n_local:(i + 1) * n_local] = np.asarray(
            res.results[i]["out"], dtype=np.float32).reshape(n_local, 256, H, W)
    return out
